# revision 11
# baseline (speedup 1.0000x reference)
"""DotLoss kernel for Trainium2, data-parallel over 8 NeuronCores.

loss = mean_i[ relu(1 + dot(img[I[i]], aud[i]) - dot(img[i], aud[i]))
             + relu(1 + dot(img[i], aud[A[i]]) - dot(img[i], aud[i])) ]

Strategy: the host pre-gathers impostor rows (img[I], aud[A]) and downcasts
the four per-core streams (img_loc, img_gat, aud_loc, aud_gat) to fp8-e4m3
(loss rel-err ~1.3e-3 vs the 2e-2 gate), uploaded TRANSPOSED with the
feature dim D on partitions. All row-dot multiplies run on the TensorEngine:
for each 128-row group g and each 128-wide D-chunk dc,

    pa[:, gi, :] += img_loc_g.T @ aud_loc_g     (anchor dots on the diag)
    pm[:, gi, :] += img_loc_g.T @ aud_gat_g     (aimp)
    pi[:, gi, :] += img_gat_g.T @ aud_loc_g     (iimp)

accumulating fp32 in PSUM, with four groups packed per PSUM bank tile
[128, 4, 128]. Diagonals are then extracted with standard DVE/ACT ops only
(the fancy fused ops are either 1x-slow or broken here: scalar_tensor_tensor
and tensor_scalar+accum run at 1 elem/cycle/lane, and the custom-ISA
tensor_tensor_reduce / tensor_mask_reduce hang the device): one
tensor_tensor multiply against a replicated identity mask (PSUM src, FD=512
amortizes the PSUM access latency) writes the masked blocks to SBUF, then
either one segmented tensor_reduce [128,4,128]->[128,4] on the DVE or four
activation-Copy-with-accum ops on the Scalar engine (split to balance the
two engines) produce per-row dot columns. A tiny hinge epilogue yields a
[128,1] partial per core; the host sums and divides by N.
"""

import numpy as np

N, D = 32768, 512
NCORES = 8
SHARD = N // NCORES          # 4096 rows per core
P = 128
KC = D // P                  # 4 contraction chunks of 128
NG = SHARD // P              # 32 groups of 128 rows
SG = 4                       # groups per supergroup (PSUM bank packing)
NSG = NG // SG               # 8 supergroups of 512 rows
# Row-block sizes (rows) for DMA chunking; multiples of 512 (supergroup).
RBS = (512, 512, 1024, 1024, 512, 512)
assert sum(RBS) == SHARD and all(r % (SG * P) == 0 for r in RBS)

_CACHE = {}


def _build_nc():
    import concourse.bacc as bacc
    import concourse.mybir as mybir
    import concourse.tile as tile
    from contextlib import ExitStack

    fp32 = mybir.dt.float32
    fp8 = mybir.dt.float8e4

    mult = mybir.AluOpType.mult
    add = mybir.AluOpType.add
    amax = mybir.AluOpType.max
    subtract = mybir.AluOpType.subtract
    copyf = mybir.ActivationFunctionType.Copy

    nc = bacc.Bacc("TRN2")
    # x[dc] per partition dk: concat over row-blocks rb of
    # [4 streams (img_loc, img_gat, aud_loc, aud_gat)] x [R rows], fp8.
    x_d = nc.dram_tensor("x", [KC, P, 4 * SHARD], fp8, kind="ExternalInput")
    eye_d = nc.dram_tensor("eye8", [P, 2 * SG, P], fp32, kind="ExternalInput")
    partial = nc.dram_tensor("partial", [P, 1], fp32, kind="ExternalOutput")

    with ExitStack() as ctx:
        tc = ctx.enter_context(tile.TileContext(nc))
        strm = ctx.enter_context(tc.tile_pool(name="strm", bufs=2))
        pai_p = ctx.enter_context(tc.tile_pool(name="pai", bufs=2, space="PSUM"))
        pm_p = ctx.enter_context(tc.tile_pool(name="pm", bufs=2, space="PSUM"))
        scr = ctx.enter_context(tc.tile_pool(name="scr", bufs=4))
        dump = ctx.enter_context(tc.tile_pool(name="dump", bufs=3))
        acc = ctx.enter_context(tc.tile_pool(name="acc", bufs=1))

        # Trigger the ACT function-table load (~2.7us) during DMA warmup.
        warm = acc.tile([P, 1], fp32, tag="warm")
        nc.vector.memset(warm[:], 0.0)
        nc.scalar.activation(out=warm[:], in_=warm[:], func=copyf)

        eye8 = acc.tile([P, 2 * SG, P], fp32, tag="eye8")
        nc.sync.dma_start(out=eye8[:], in_=eye_d[:])

        anchor = acc.tile([P, NG], fp32, tag="anchor")
        iimp = acc.tile([P, NG], fp32, tag="iimp")
        aimp = acc.tile([P, NG], fp32, tag="aimp")

        def act_reduce(src, dst_col):
            du = dump.tile([P, P], fp32, tag="dump")
            nc.scalar.activation(
                out=du[:], in_=src, func=copyf, accum_out=dst_col)

        r0 = 0
        sg = 0
        for R in RBS:
            tiles = []
            for dc in range(KC):
                t = strm.tile([P, 4, R], fp8, tag=f"x{dc}")
                nc.sync.dma_start(
                    out=t[:],
                    in_=x_d[dc, :, 4 * r0:4 * (r0 + R)].rearrange(
                        "p (s r) -> p s r", s=4),
                )
                tiles.append(t)
            for sgl in range(R // (SG * P)):
                # pai block [sg-group][0]=anchor, [1]=iimp; pm = aimp
                pai = pai_p.tile([P, SG, 2, P], fp32, tag="pai")
                pm = pm_p.tile([P, SG, P], fp32, tag="pm")
                for gi in range(SG):
                    l0 = (sgl * SG + gi) * P
                    for dc in range(KC):
                        t = tiles[dc]
                        li = t[:, 0, l0:l0 + P]
                        la = t[:, 2, l0:l0 + P]
                        ga = t[:, 3, l0:l0 + P]
                        lig = t[:, 0:2, l0:l0 + P]
                        st = dict(start=(dc == 0), stop=(dc == KC - 1))
                        # stationary la serves anchor+iimp in one n=256 mm
                        nc.tensor.matmul(pai[:, gi, :, :], la, lig, **st)
                        nc.tensor.matmul(pm[:, gi, :], ga, li, **st)
                # Diag extraction, standard ops only: mask-multiply from PSUM
                # on DVE, then segmented tensor_reduce (DVE) or activation
                # Copy+accum (ACT), split to balance the engines.
                o_ai = scr.tile([P, SG, 2, P], fp32, tag="o_ai")
                nc.vector.tensor_tensor(
                    out=o_ai[:], in0=pai[:],
                    in1=eye8[:].rearrange("p (a b) g -> p a b g", b=2),
                    op=mult)
                o_m = scr.tile([P, SG, P], fp32, tag="o_m")
                nc.vector.tensor_tensor(
                    out=o_m[:], in0=pm[:], in1=eye8[:, 0:SG, :], op=mult)
                c0 = SG * sg
                c1 = SG * (sg + 1)
                if sg % 2 == 0:
                    nc.vector.tensor_reduce(
                        out=anchor[:, c0:c1], in_=o_ai[:, :, 0, :],
                        axis=mybir.AxisListType.X, op=add)
                    nc.vector.tensor_reduce(
                        out=iimp[:, c0:c1], in_=o_ai[:, :, 1, :],
                        axis=mybir.AxisListType.X, op=add)
                else:
                    for gi in range(SG):
                        g = SG * sg + gi
                        act_reduce(o_ai[:, gi, 0, :], anchor[:, g:g + 1])
                        act_reduce(o_ai[:, gi, 1, :], iimp[:, g:g + 1])
                for gi in range(SG):
                    g = SG * sg + gi
                    act_reduce(o_m[:, gi, :], aimp[:, g:g + 1])
                sg += 1
            r0 += R

        diff = acc.tile([P, 2 * NG], fp32, tag="diff")
        nc.vector.tensor_tensor(
            out=diff[:, 0:NG], in0=iimp[:], in1=anchor[:], op=subtract)
        nc.vector.tensor_tensor(
            out=diff[:, NG:], in0=aimp[:], in1=anchor[:], op=subtract)
        hout = acc.tile([P, 2 * NG], fp32, tag="hout")
        nc.vector.tensor_scalar(
            out=hout[:], in0=diff[:], scalar1=1.0, scalar2=0.0,
            op0=add, op1=amax,
        )
        psum_t = acc.tile([P, 1], fp32, tag="psum_t")
        nc.vector.tensor_reduce(
            out=psum_t[:], in_=hout[:], axis=mybir.AxisListType.X, op=add,
        )
        nc.sync.dma_start(out=partial[:], in_=psum_t[:])

    nc.compile()
    return nc


def _get_nc():
    if "nc" not in _CACHE:
        _CACHE["nc"] = _build_nc()
    return _CACHE["nc"]


def make_in_maps(image_outputs, audio_outputs, I_imp_ind, A_imp_ind):
    import ml_dtypes

    fp8 = ml_dtypes.float8_e4m3
    img = np.asarray(image_outputs, dtype=np.float32).astype(fp8)
    aud = np.asarray(audio_outputs, dtype=np.float32).astype(fp8)
    I_imp = np.asarray(I_imp_ind).astype(np.int64)
    A_imp = np.asarray(A_imp_ind).astype(np.int64)

    def tr(a):
        return np.ascontiguousarray(a.T).reshape(KC, P, N)

    sT = [tr(img), tr(img[I_imp]), tr(aud), tr(aud[A_imp])]
    eye8 = np.broadcast_to(
        np.eye(P, dtype=np.float32)[:, None, :], (P, 2 * SG, P)
    ).copy()
    in_maps = []
    for c in range(NCORES):
        b = c * SHARD
        x = np.empty((KC, P, 4 * SHARD), dtype=fp8)
        r0 = 0
        for R in RBS:
            blk = np.stack(
                [s[:, :, b + r0:b + r0 + R] for s in sT], axis=2
            )  # [KC, P, 4, R]
            x[:, :, 4 * r0:4 * (r0 + R)] = blk.reshape(KC, P, 4 * R)
            r0 += R
        in_maps.append({"x": x, "eye8": eye8})
    return in_maps


def kernel(image_outputs, audio_outputs, I_imp_ind, A_imp_ind):
    from concourse import bass_utils

    nc = _get_nc()
    in_maps = make_in_maps(image_outputs, audio_outputs, I_imp_ind, A_imp_ind)
    res = bass_utils.run_bass_kernel_spmd(nc, in_maps, list(range(NCORES))).results
    total = sum(float(r["partial"].sum(dtype=np.float64)) for r in res)
    return np.float32(total / N)


# revision 12
# speedup vs baseline: 1.1143x; 1.1143x over previous
"""DotLoss kernel for Trainium2, data-parallel over 8 NeuronCores.

loss = mean_i[ relu(1 + dot(img[I[i]], aud[i]) - dot(img[i], aud[i]))
             + relu(1 + dot(img[i], aud[A[i]]) - dot(img[i], aud[i])) ]

Strategy: the host pre-gathers impostor rows (img[I], aud[A]) and downcasts
the four per-core streams (img_loc, img_gat, aud_loc, aud_gat) to fp8-e4m3
(loss rel-err ~1.3e-3 vs the 2e-2 gate), uploaded TRANSPOSED with the
feature dim D on partitions. The whole 8MB shard fits in SBUF (64KB per
partition), so all 32 stream DMAs (8 row-blocks x 4 D-chunks, 256KB each)
are issued up front with no buffer reuse -- they drain in issue order at
full bandwidth while compute chases block by block.

All row-dot multiplies run on the TensorEngine. For each 128-row group and
D-chunk dc, with aud_loc as the stationary operand one n=256 matmul yields
both anchor and iimp diag-blocks (rhs = [img_loc | img_gat]), and a second
n=128 matmul (stationary aud_gat, rhs img_loc) yields aimp:

    pai[:, gi, 0:2, :] += aud_loc_g.T @ [img_loc_g | img_gat_g]
    pm[:, gi, :]       += aud_gat_g.T @ img_loc_g

accumulated fp32 in PSUM with four groups per PSUM tile. Diagonals are
extracted with standard ops only (fused reduce ops are 1x-slow or broken
here: scalar_tensor_tensor / tensor_scalar+accum run at 1 elem/cycle/lane;
custom-ISA tensor_tensor_reduce / tensor_mask_reduce hang the device): one
DVE tensor_tensor multiply against a replicated fp8 identity mask (PSUM
src, big FD amortizes PSUM latency) writes masked blocks to SBUF, then
segmented tensor_reduce [128,4,128]->[128,4] on the DVE or activation
Copy+accum on the Scalar engine (split to balance engines) produce per-row
dot columns. A small hinge epilogue emits a [128,1] partial per core; the
host sums the 8 partials and divides by N.
"""

import numpy as np

N, D = 32768, 512
NCORES = 8
SHARD = N // NCORES          # 4096 rows per core
P = 128
KC = D // P                  # 4 contraction chunks of 128
NG = SHARD // P              # 32 groups of 128 rows
SG = 4                       # groups per supergroup (PSUM tile packing)
NSG = NG // SG               # 8 supergroups of 512 rows
RB = SG * P                  # 512 rows per row-block (= one supergroup)

_CACHE = {}


def _build_nc():
    import concourse.bacc as bacc
    import concourse.mybir as mybir
    import concourse.tile as tile
    from contextlib import ExitStack

    fp32 = mybir.dt.float32
    fp8 = mybir.dt.float8e4

    mult = mybir.AluOpType.mult
    add = mybir.AluOpType.add
    amax = mybir.AluOpType.max
    subtract = mybir.AluOpType.subtract
    copyf = mybir.ActivationFunctionType.Copy

    nc = bacc.Bacc("TRN2")
    # x[dc][dk]: [8 row-blocks][4 streams (li, gi, la, ga)][512 rows] fp8
    x_d = nc.dram_tensor("x", [KC, P, NSG, 4, RB], fp8, kind="ExternalInput")
    eye_d = nc.dram_tensor("eye8", [P, 2 * SG, P], fp8, kind="ExternalInput")
    partial = nc.dram_tensor("partial", [P, 1], fp32, kind="ExternalOutput")

    with ExitStack() as ctx:
        tc = ctx.enter_context(tile.TileContext(nc))
        strm = ctx.enter_context(tc.tile_pool(name="strm", bufs=1))
        pai_p = ctx.enter_context(tc.tile_pool(name="pai", bufs=3, space="PSUM"))
        pm_p = ctx.enter_context(tc.tile_pool(name="pm", bufs=2, space="PSUM"))
        scr = ctx.enter_context(tc.tile_pool(name="scr", bufs=6))
        dump = ctx.enter_context(tc.tile_pool(name="dump", bufs=4))
        acc = ctx.enter_context(tc.tile_pool(name="acc", bufs=1))

        # Trigger the ACT function-table load (~2.7us) during DMA warmup.
        warm = acc.tile([P, 1], fp32, tag="warm")
        nc.vector.memset(warm[:], 0.0)
        nc.scalar.activation(out=warm[:], in_=warm[:], func=copyf)

        eye8 = acc.tile([P, 2 * SG, P], fp8, tag="eye8")
        nc.sync.dma_start(out=eye8[:], in_=eye_d[:])

        anchor = acc.tile([P, NG], fp32, tag="anchor")
        iimp = acc.tile([P, NG], fp32, tag="iimp")
        aimp = acc.tile([P, NG], fp32, tag="aimp")

        def act_reduce(src, dst_col):
            du = dump.tile([P, P], fp32, tag="dump")
            nc.scalar.activation(
                out=du[:], in_=src, func=copyf, accum_out=dst_col)

        # whole-shard stream tiles; DMAs issue in row-block order and drain
        # without any WAR gating
        xt = []
        for dc in range(KC):
            t = strm.tile([P, NSG, 4, RB], fp8, tag=f"x{dc}")
            xt.append(t)
        for sg in range(NSG):
            for dc in range(KC):
                nc.sync.dma_start(
                    out=xt[dc][:, sg, :, :], in_=x_d[dc, :, sg, :, :])

        for sg in range(NSG):
            pai = pai_p.tile([P, SG, 2, P], fp32, tag="pai")
            pm = pm_p.tile([P, SG, P], fp32, tag="pm")
            for gi in range(SG):
                l0 = gi * P
                for dc in range(KC):
                    t = xt[dc]
                    li = t[:, sg, 0, l0:l0 + P]
                    la = t[:, sg, 2, l0:l0 + P]
                    ga = t[:, sg, 3, l0:l0 + P]
                    lig = t[:, sg, 0:2, l0:l0 + P]
                    st = dict(start=(dc == 0), stop=(dc == KC - 1))
                    nc.tensor.matmul(pai[:, gi, :, :], la, lig, **st)
                    nc.tensor.matmul(pm[:, gi, :], ga, li, **st)
            o_ai = scr.tile([P, SG, 2, P], fp32, tag="o_ai")
            nc.vector.tensor_tensor(
                out=o_ai[:], in0=pai[:],
                in1=eye8[:].rearrange("p (a b) g -> p a b g", b=2),
                op=mult)
            o_m = scr.tile([P, SG, P], fp32, tag="o_m")
            nc.vector.tensor_tensor(
                out=o_m[:], in0=pm[:], in1=eye8[:, 0:SG, :], op=mult)
            c0, c1 = SG * sg, SG * (sg + 1)
            if sg % 2 == 0:
                nc.vector.tensor_reduce(
                    out=anchor[:, c0:c1], in_=o_ai[:, :, 0, :],
                    axis=mybir.AxisListType.X, op=add)
                nc.vector.tensor_reduce(
                    out=iimp[:, c0:c1], in_=o_ai[:, :, 1, :],
                    axis=mybir.AxisListType.X, op=add)
            else:
                for gi in range(SG):
                    g = SG * sg + gi
                    act_reduce(o_ai[:, gi, 0, :], anchor[:, g:g + 1])
                    act_reduce(o_ai[:, gi, 1, :], iimp[:, g:g + 1])
            for gi in range(SG):
                g = SG * sg + gi
                act_reduce(o_m[:, gi, :], aimp[:, g:g + 1])

        diff = acc.tile([P, 2 * NG], fp32, tag="diff")
        nc.vector.tensor_tensor(
            out=diff[:, 0:NG], in0=iimp[:], in1=anchor[:], op=subtract)
        nc.vector.tensor_tensor(
            out=diff[:, NG:], in0=aimp[:], in1=anchor[:], op=subtract)
        hout = acc.tile([P, 2 * NG], fp32, tag="hout")
        nc.vector.tensor_scalar(
            out=hout[:], in0=diff[:], scalar1=1.0, scalar2=0.0,
            op0=add, op1=amax,
        )
        psum_t = acc.tile([P, 1], fp32, tag="psum_t")
        nc.vector.tensor_reduce(
            out=psum_t[:], in_=hout[:], axis=mybir.AxisListType.X, op=add,
        )
        nc.sync.dma_start(out=partial[:], in_=psum_t[:])

    nc.compile()
    return nc


def _get_nc():
    if "nc" not in _CACHE:
        _CACHE["nc"] = _build_nc()
    return _CACHE["nc"]


def make_in_maps(image_outputs, audio_outputs, I_imp_ind, A_imp_ind):
    import ml_dtypes

    fp8 = ml_dtypes.float8_e4m3
    img = np.asarray(image_outputs, dtype=np.float32).astype(fp8)
    aud = np.asarray(audio_outputs, dtype=np.float32).astype(fp8)
    I_imp = np.asarray(I_imp_ind).astype(np.int64)
    A_imp = np.asarray(A_imp_ind).astype(np.int64)

    def tr(a):
        return np.ascontiguousarray(a.T).reshape(KC, P, N)

    sT = [tr(img), tr(img[I_imp]), tr(aud), tr(aud[A_imp])]
    eye8 = np.broadcast_to(
        np.eye(P, dtype=np.float32)[:, None, :], (P, 2 * SG, P)
    ).astype(fp8)
    in_maps = []
    for c in range(NCORES):
        b = c * SHARD
        # [KC, P, NSG, 4, RB]
        x = np.stack(
            [s[:, :, b:b + SHARD].reshape(KC, P, NSG, RB) for s in sT],
            axis=3,
        )
        in_maps.append({"x": np.ascontiguousarray(x), "eye8": eye8})
    return in_maps


def kernel(image_outputs, audio_outputs, I_imp_ind, A_imp_ind):
    from concourse import bass_utils

    nc = _get_nc()
    in_maps = make_in_maps(image_outputs, audio_outputs, I_imp_ind, A_imp_ind)
    res = bass_utils.run_bass_kernel_spmd(nc, in_maps, list(range(NCORES))).results
    total = sum(float(r["partial"].sum(dtype=np.float64)) for r in res)
    return np.float32(total / N)


# revision 13
# speedup vs baseline: 1.2896x; 1.1574x over previous
"""DotLoss kernel for Trainium2, data-parallel over 8 NeuronCores.

loss = mean_i[ relu(1 + dot(img[I[i]], aud[i]) - dot(img[i], aud[i]))
             + relu(1 + dot(img[i], aud[A[i]]) - dot(img[i], aud[i])) ]

Strategy: the host pre-gathers impostor rows (img[I], aud[A]) and downcasts
the four per-core streams (img_loc, img_gat, aud_loc, aud_gat) to fp8-e4m3
(loss rel-err ~1.3e-3 vs the 2e-2 gate), uploaded TRANSPOSED with the
feature dim D on partitions. The whole 8MB shard fits in SBUF (64KB per
partition), so all 32 stream DMAs (8 row-blocks x 4 D-chunks, 256KB each)
are issued up front with no buffer reuse -- they drain in issue order at
full bandwidth while compute chases block by block.

All row-dot multiplies run on the TensorEngine. For each 128-row group and
D-chunk dc, with aud_loc as the stationary operand one n=256 matmul yields
both anchor and iimp diag-blocks (rhs = [img_loc | img_gat]), and a second
n=128 matmul (stationary aud_gat, rhs img_loc) yields aimp:

    pai[:, gi, 0:2, :] += aud_loc_g.T @ [img_loc_g | img_gat_g]
    pm[:, gi, :]       += aud_gat_g.T @ img_loc_g

accumulated fp32 in PSUM with four groups per PSUM tile. Diagonals are
extracted with standard ops only (fused reduce ops are 1x-slow or broken
here: scalar_tensor_tensor / tensor_scalar+accum run at 1 elem/cycle/lane;
custom-ISA tensor_tensor_reduce / tensor_mask_reduce hang the device): one
DVE tensor_tensor multiply against a replicated fp8 identity mask (PSUM
src, big FD amortizes PSUM latency) writes masked blocks to SBUF, then
segmented tensor_reduce [128,4,128]->[128,4] on the DVE or activation
Copy+accum on the Scalar engine (split to balance engines) produce per-row
dot columns. A small hinge epilogue emits a [128,1] partial per core; the
host sums the 8 partials and divides by N.
"""

import numpy as np

N, D = 32768, 512
NCORES = 8
SHARD = N // NCORES          # 4096 rows per core
P = 128
KC = D // P                  # 4 contraction chunks of 128
NG = SHARD // P              # 32 groups of 128 rows
SG = 4                       # groups per supergroup (PSUM tile packing)
NSG = NG // SG               # 8 supergroups of 512 rows
RB = SG * P                  # 512 rows per row-block (= one supergroup)

_CACHE = {}


def _build_nc():
    import concourse.bacc as bacc
    import concourse.mybir as mybir
    import concourse.tile as tile
    from contextlib import ExitStack

    fp32 = mybir.dt.float32
    fp8 = mybir.dt.float8e4

    mult = mybir.AluOpType.mult
    add = mybir.AluOpType.add
    amax = mybir.AluOpType.max
    subtract = mybir.AluOpType.subtract
    copyf = mybir.ActivationFunctionType.Copy

    nc = bacc.Bacc("TRN2")
    # x[dc][dk]: [8 row-blocks][4 streams (li, gi, la, ga)][512 rows] fp8
    x_d = nc.dram_tensor("x", [KC, P, NSG, 4, RB], fp8, kind="ExternalInput")
    eye_d = nc.dram_tensor("eye8", [P, 2 * SG, P], fp8, kind="ExternalInput")
    partial = nc.dram_tensor("partial", [P, 1], fp32, kind="ExternalOutput")

    with ExitStack() as ctx:
        tc = ctx.enter_context(tile.TileContext(nc))
        strm = ctx.enter_context(tc.tile_pool(name="strm", bufs=1))
        pai_p = ctx.enter_context(tc.tile_pool(name="pai", bufs=3, space="PSUM"))
        pm_p = ctx.enter_context(tc.tile_pool(name="pm", bufs=2, space="PSUM"))
        scr = ctx.enter_context(tc.tile_pool(name="scr", bufs=6))
        dump = ctx.enter_context(tc.tile_pool(name="dump", bufs=4))
        acc = ctx.enter_context(tc.tile_pool(name="acc", bufs=1))

        # Trigger the ACT function-table load (~2.7us) during DMA warmup.
        warm = acc.tile([P, 1], fp32, tag="warm")
        nc.vector.memset(warm[:], 0.0)
        nc.scalar.activation(out=warm[:], in_=warm[:], func=copyf)

        eye8 = acc.tile([P, 2 * SG, P], fp8, tag="eye8")
        nc.sync.dma_start(out=eye8[:], in_=eye_d[:])

        anchor = acc.tile([P, NG], fp32, tag="anchor")
        iimp = acc.tile([P, NG], fp32, tag="iimp")
        aimp = acc.tile([P, NG], fp32, tag="aimp")

        def act_reduce(src, dst_col):
            du = dump.tile([P, P], fp32, tag="dump")
            nc.scalar.activation(
                out=du[:], in_=src, func=copyf, accum_out=dst_col)

        # whole-shard stream tiles; DMAs issue in row-block order and drain
        # without any WAR gating
        xt = []
        for dc in range(KC):
            t = strm.tile([P, NSG, 4, RB], fp8, tag=f"x{dc}")
            xt.append(t)
        for sg in range(NSG):
            for dc in range(KC):
                nc.sync.dma_start(
                    out=xt[dc][:, sg, :, :], in_=x_d[dc, :, sg, :, :])

        for sg in range(NSG):
            pai = pai_p.tile([P, SG, 2, P], fp32, tag="pai")
            pm = pm_p.tile([P, SG, P], fp32, tag="pm")
            for gi in range(SG):
                l0 = gi * P
                for dc in range(KC):
                    t = xt[dc]
                    li = t[:, sg, 0, l0:l0 + P]
                    la = t[:, sg, 2, l0:l0 + P]
                    ga = t[:, sg, 3, l0:l0 + P]
                    lig = t[:, sg, 0:2, l0:l0 + P]
                    st = dict(start=(dc == 0), stop=(dc == KC - 1))
                    nc.tensor.matmul(pai[:, gi, :, :], la, lig, **st)
                    nc.tensor.matmul(pm[:, gi, :], ga, li, **st)
            o_ai = scr.tile([P, SG, 2, P], fp32, tag="o_ai")
            nc.vector.tensor_tensor(
                out=o_ai[:], in0=pai[:],
                in1=eye8[:].rearrange("p (a b) g -> p a b g", b=2),
                op=mult)
            o_m = scr.tile([P, SG, P], fp32, tag="o_m")
            nc.vector.tensor_tensor(
                out=o_m[:], in0=pm[:], in1=eye8[:, 0:SG, :], op=mult)
            c0, c1 = SG * sg, SG * (sg + 1)
            nc.vector.tensor_reduce(
                out=anchor[:, c0:c1], in_=o_ai[:, :, 0, :],
                axis=mybir.AxisListType.X, op=add)
            nc.vector.tensor_reduce(
                out=iimp[:, c0:c1], in_=o_ai[:, :, 1, :],
                axis=mybir.AxisListType.X, op=add)
            for gi in range(SG):
                g = SG * sg + gi
                act_reduce(o_m[:, gi, :], aimp[:, g:g + 1])

        diff = acc.tile([P, 2 * NG], fp32, tag="diff")
        nc.vector.tensor_tensor(
            out=diff[:, 0:NG], in0=iimp[:], in1=anchor[:], op=subtract)
        nc.vector.tensor_tensor(
            out=diff[:, NG:], in0=aimp[:], in1=anchor[:], op=subtract)
        hout = acc.tile([P, 2 * NG], fp32, tag="hout")
        nc.vector.tensor_scalar(
            out=hout[:], in0=diff[:], scalar1=1.0, scalar2=0.0,
            op0=add, op1=amax,
        )
        psum_t = acc.tile([P, 1], fp32, tag="psum_t")
        nc.vector.tensor_reduce(
            out=psum_t[:], in_=hout[:], axis=mybir.AxisListType.X, op=add,
        )
        nc.sync.dma_start(out=partial[:], in_=psum_t[:])

    nc.compile()
    return nc


def _get_nc():
    if "nc" not in _CACHE:
        _CACHE["nc"] = _build_nc()
    return _CACHE["nc"]


def make_in_maps(image_outputs, audio_outputs, I_imp_ind, A_imp_ind):
    import ml_dtypes

    fp8 = ml_dtypes.float8_e4m3
    img = np.asarray(image_outputs, dtype=np.float32).astype(fp8)
    aud = np.asarray(audio_outputs, dtype=np.float32).astype(fp8)
    I_imp = np.asarray(I_imp_ind).astype(np.int64)
    A_imp = np.asarray(A_imp_ind).astype(np.int64)

    def tr(a):
        return np.ascontiguousarray(a.T).reshape(KC, P, N)

    sT = [tr(img), tr(img[I_imp]), tr(aud), tr(aud[A_imp])]
    eye8 = np.broadcast_to(
        np.eye(P, dtype=np.float32)[:, None, :], (P, 2 * SG, P)
    ).astype(fp8)
    in_maps = []
    for c in range(NCORES):
        b = c * SHARD
        # [KC, P, NSG, 4, RB]
        x = np.stack(
            [s[:, :, b:b + SHARD].reshape(KC, P, NSG, RB) for s in sT],
            axis=3,
        )
        in_maps.append({"x": np.ascontiguousarray(x), "eye8": eye8})
    return in_maps


def kernel(image_outputs, audio_outputs, I_imp_ind, A_imp_ind):
    from concourse import bass_utils

    nc = _get_nc()
    in_maps = make_in_maps(image_outputs, audio_outputs, I_imp_ind, A_imp_ind)
    res = bass_utils.run_bass_kernel_spmd(nc, in_maps, list(range(NCORES))).results
    total = sum(float(r["partial"].sum(dtype=np.float64)) for r in res)
    return np.float32(total / N)
